# revision 25
# baseline (speedup 1.0000x reference)
"""Multihead attention (B=2, L=2048, D=1024, 16 heads) on 8 trn2 cores.

Sharding: tensor-parallel over heads — 2 heads per core. Each core computes
q/k/v projections for its 128 columns of Wq/Wk/Wv, full attention for its two
heads, and a partial output projection against its 128 rows of Wo. The host
sums the 8 partials and adds bo.

Engine budget per core: ScalarE exp over B*HPC*L^2 = 16.8M scores is ~147us
of ACTIVATE; PE is ~170us of bf16 matmul. The schedule keeps both dense:

  - Warmup matmuls at kernel start trip the PE HAM clock-gate out of its
    cold 1.2GHz state while the first x tiles stream in (DMA spread over
    sync/scalar/vector queues; weight tensors on gpsimd/vector/scalar so
    wv is not serialized behind wq/wk).
  - Head phase: batch-0 projections are software-pipelined with the first
    l-chunk's attention, and score pairs of LATER l-chunks are "stolen" into
    the head (exps parked in SBUF) so ScalarE never starves while the PE
    grinds projections. Chunk-0 q/k bias-adds run on ScalarE (idle then).
  - scoresT packs both heads into one PE pass via 64-row tile_position;
    attn@v accumulates both s-tiles of a pair in PSUM with the softmax
    denominator as a 65th ones-column of v.
  - o-projection of the previous chunk is spread one l-tile at a time
    between score pairs (a single block starves ScalarE via PSUM-pool
    rotation); batch-1 projections fill batch-0's chunks the same way.
  - The tail normalize uses a PE broadcast (PE idle there) straight from
    attn@v PSUM.
"""

from contextlib import ExitStack

import ml_dtypes
import numpy as np

import concourse.bacc as bacc
import concourse.mybir as mybir
import concourse.tile as tile
from concourse.bass_utils import run_bass_kernel_spmd

D_MODEL = 1024
N_HEAD = 16
HEAD_DIM = 64
B = 2
L = 2048
N_CORES = 8
HPC = N_HEAD // N_CORES  # heads per core
MLOC = HPC * HEAD_DIM  # 128: local d width per core

F32 = mybir.dt.float32
BF16 = mybir.dt.bfloat16
NPBF16 = ml_dtypes.bfloat16


def build_nc(Lb=L, lc_size=512, nch=512):
    """Build the per-core Bass program. Lb = sequence length per batch."""
    BLb = B * Lb
    KT = D_MODEL // 128  # 8 contraction tiles for the projections
    n_nch = BLb // nch  # projection column chunks (8; 0-3 = b0, 4-7 = b1)
    n_lc = Lb // lc_size  # attention l-chunks per batch
    n_st = Lb // 128  # s-tiles per batch
    n_pair = n_st // 2  # s-tile pairs per batch

    nc = bacc.Bacc("TRN2", target_bir_lowering=False, debug=False)

    xT = nc.dram_tensor("xT", [D_MODEL, BLb], BF16, kind="ExternalInput").ap()
    wq = nc.dram_tensor("wq", [D_MODEL, MLOC], BF16, kind="ExternalInput").ap()
    wk = nc.dram_tensor("wk", [D_MODEL, MLOC], BF16, kind="ExternalInput").ap()
    wv = nc.dram_tensor("wv", [D_MODEL, MLOC], BF16, kind="ExternalInput").ap()
    wo = nc.dram_tensor("wo", [MLOC, D_MODEL], BF16, kind="ExternalInput").ap()
    bq = nc.dram_tensor("bq", [MLOC, 1], F32, kind="ExternalInput").ap()
    bk = nc.dram_tensor("bk", [MLOC, 1], F32, kind="ExternalInput").ap()
    bv = nc.dram_tensor("bv", [MLOC, 1], F32, kind="ExternalInput").ap()
    out = nc.dram_tensor("out", [BLb, D_MODEL], F32, kind="ExternalOutput").ap()

    with tile.TileContext(nc) as tc, ExitStack() as ctx:
        consts = ctx.enter_context(tc.tile_pool(name="consts", bufs=1))
        qk_sb = ctx.enter_context(tc.tile_pool(name="qk_sb", bufs=1))
        xt_pool = ctx.enter_context(tc.tile_pool(name="xt", bufs=32))
        # PSUM: big pool (2-bank slots x3) shared by scoresT / projections /
        # o-proj; av pool: one 1-bank tile per head. Total 8 banks.
        big_ps = ctx.enter_context(tc.tile_pool(name="big_ps", bufs=3, space="PSUM"))
        av_ps = ctx.enter_context(tc.tile_pool(name="av_ps", bufs=1, space="PSUM"))
        exp_pool = ctx.enter_context(tc.tile_pool(name="expT", bufs=4))
        pre_pool = ctx.enter_context(tc.tile_pool(name="preT", bufs=1))
        att_sb = ctx.enter_context(tc.tile_pool(name="att_sb", bufs=3))
        out_pool = ctx.enter_context(tc.tile_pool(name="out_sb", bufs=6))

        # PE warmup: ~4us of junk matmuls (no input deps) so the HAM clock
        # gate reaches 2.4GHz before the first real projection matmuls.
        warm = consts.tile([1, 128], F32, tag="warm")
        nc.vector.memset(warm[:], 0.0)
        ps_warm = big_ps.tile([128, 2, nch], F32, tag="big", name="ps_warm")
        for _ in range(10):
            nc.tensor.matmul(ps_warm[:, 0, :128], warm[:], warm[:],
                             start=True, stop=True)

        def load_xts(nc_i, first=False):
            csl = slice(nc_i * nch, (nc_i + 1) * nch)
            xts = []
            for k in range(KT):
                xt = xt_pool.tile([128, nch], BF16, tag="xt", name="xt")
                # scalar-queue DMAs steal ScalarE issue slots from ACTIVATE;
                # only the first chunk (pre-exp) may use the scalar queue
                eng = (nc.sync if k % 2 == 0 else nc.scalar) if first else nc.sync
                eng.dma_start(xt[:], xT[128 * k : 128 * (k + 1), csl])
                xts.append(xt)
            return xts

        # First projection chunk's x tiles before anything else so the PE
        # starts as early as possible.
        xts0 = load_xts(0, first=True)

        # Weights + biases spread over the three DMA queues so nothing that
        # gates the first scores (bq/bk!) or first attn@v (wv) queues behind
        # the bulk weight transfers: gpsimd = biases + wq + wo, sync = x
        # evens + wk, scalar = x odds + wv.
        bq_sb = consts.tile([MLOC, 1], F32, tag="bq")
        bk_sb = consts.tile([MLOC, 1], F32, tag="bk")
        bv_sb = consts.tile([MLOC, 1], F32, tag="bv")
        for b_sb, b_dram in ((bq_sb, bq), (bk_sb, bk), (bv_sb, bv)):
            nc.gpsimd.dma_start(b_sb[:], b_dram)
        wq_sb = consts.tile([128, KT, MLOC], BF16, tag="wq")
        wk_sb = consts.tile([128, KT, MLOC], BF16, tag="wk")
        wv_sb = consts.tile([128, KT, MLOC], BF16, tag="wv")
        wo_sb = consts.tile([128, D_MODEL], BF16, tag="wo")
        for eng, w_sb, w_dram in ((nc.gpsimd, wq_sb, wq), (nc.sync, wk_sb, wk),
                                  (nc.scalar, wv_sb, wv)):
            wr = w_dram.rearrange("(k p) m -> p k m", p=128)
            for k in range(KT):
                eng.dma_start(w_sb[:, k, :], wr[:, k, :])
        nc.gpsimd.dma_start(wo_sb[:], wo)
        ones_f32 = consts.tile([1, 128], F32, tag="ones_f32")
        nc.vector.memset(ones_f32[:], 1.0)

        # Persistent activations.
        qT_sb = qk_sb.tile([128, BLb], BF16, tag="qT")  # [d_local, b*Lb+l]
        kT_sb = qk_sb.tile([128, BLb], BF16, tag="kT")
        # v (natural layout) + ones column: per (b, head): [128, n_st, 65]
        vaug = [
            [qk_sb.tile([128, n_st, HEAD_DIM + 1], BF16, tag=f"vaug{bi}{h}",
                        name=f"vaug{bi}{h}")
             for h in range(HPC)]
            for bi in range(B)
        ]
        for bi in range(B):
            for h in range(HPC):
                nc.vector.memset(vaug[bi][h][:, :, HEAD_DIM:], 1.0)

        def qk_proj(nc_i, xts=None, scalar_bias=False):
            """q/k projections for one column chunk of x. Returns xts for the
            later v_proj of the same chunk."""
            csl = slice(nc_i * nch, (nc_i + 1) * nch)
            if xts is None:
                xts = load_xts(nc_i)
            ps_qk = big_ps.tile([128, 2, nch], F32, tag="big", name="ps_qk")
            for k in range(KT):
                nc.tensor.matmul(ps_qk[:, 0, :], wq_sb[:, k, :], xts[k][:],
                                 start=(k == 0), stop=(k == KT - 1))
                nc.tensor.matmul(ps_qk[:, 1, :], wk_sb[:, k, :], xts[k][:],
                                 start=(k == 0), stop=(k == KT - 1))
            if scalar_bias:
                # ScalarE is idle during the head; keep DVE off the critical
                # path to the first scores.
                nc.scalar.activation(qT_sb[:, csl], ps_qk[:, 0, :],
                                     mybir.ActivationFunctionType.Identity,
                                     bias=bq_sb[:])
                nc.scalar.activation(kT_sb[:, csl], ps_qk[:, 1, :],
                                     mybir.ActivationFunctionType.Identity,
                                     bias=bk_sb[:])
            else:
                nc.vector.tensor_scalar_add(qT_sb[:, csl], ps_qk[:, 0, :],
                                            bq_sb[:])
                nc.vector.tensor_scalar_add(kT_sb[:, csl], ps_qk[:, 1, :],
                                            bk_sb[:])
            return xts

        def v_proj(nc_i, xts):
            """v projection (natural [s, d] layout) for one chunk."""
            st_per_nch = nch // 128
            ps_v = big_ps.tile([128, nch], F32, tag="big", name="ps_v")
            for st in range(st_per_nch):
                ssl = slice(128 * st, 128 * (st + 1))
                for k in range(KT):
                    nc.tensor.matmul(ps_v[:, ssl], xts[k][:, ssl],
                                     wv_sb[:, k, :],
                                     start=(k == 0), stop=(k == KT - 1))
            for st in range(st_per_nch):
                st_g = nc_i * st_per_nch + st
                bi, st_b = divmod(st_g, n_st)
                for h in range(HPC):
                    nc.vector.tensor_copy(
                        vaug[bi][h][:, st_b, :HEAD_DIM],
                        ps_v[:, 128 * st + HEAD_DIM * h
                             : 128 * st + HEAD_DIM * (h + 1)])

        def sc_pair(bi, lsl, width, t, ex):
            """scoresT + exp for s-tile pair t of one l-chunk, into bf16 ex
            tile [128, HPC, 2, lc_size]."""
            for i in range(2):
                st = 2 * t + i
                ssl = slice(bi * Lb + st * 128, bi * Lb + (st + 1) * 128)
                ps_sc = big_ps.tile([128, HPC, lc_size], F32, tag="big",
                                    name="ps_sc")
                for h in range(HPC):
                    hsl = slice(64 * h, 64 * (h + 1))
                    nc.tensor.matmul(ps_sc[:, h, :width], kT_sb[hsl, ssl],
                                     qT_sb[hsl, lsl],
                                     start=True, stop=True,
                                     tile_position=(64 * h, 0))
                nc.scalar.activation(ex[:, :, i, :width], ps_sc[:, :, :width],
                                     mybir.ActivationFunctionType.Exp,
                                     scale=1.0 / np.sqrt(HEAD_DIM))

        def av_pair(bi, ps_av, t, ex, width, start, stop):
            """attn@v for s-tile pair t (both s-tiles, both heads)."""
            for i in range(2):
                st = 2 * t + i
                for h in range(HPC):
                    nc.tensor.matmul(ps_av[h][:, :width],
                                     vaug[bi][h][:, st, :],
                                     ex[:, h, i, :width],
                                     start=start and i == 0,
                                     stop=stop and i == 1)

        def norm_part(avs_h, width, pe_bcast=False):
            """Normalization chain -> oT (bf16 lhsT for o-proj). avs_h is a
            per-head list of [65, width] APs (SBUF copies, or PSUM directly
            for the final chunk). pe_bcast uses a K=1 PE matmul for the
            denominator broadcast (for the tail, where the PE is idle)."""
            den = att_sb.tile([1, 2, lc_size], F32, tag="den", name="den")
            rcp = att_sb.tile([128, 2, lc_size], F32, tag="rcp", name="rcp")
            for h in range(HPC):
                nc.vector.tensor_copy(den[0:1, h, :width], avs_h[h][64:65, :width])
            if pe_bcast:
                ps_r = big_ps.tile([128, 2, lc_size], F32, tag="big", name="ps_r")
                for h in range(HPC):
                    nc.tensor.matmul(ps_r[:, h, :width], ones_f32[:],
                                     den[0:1, h, :width], start=True, stop=True)
                nc.vector.reciprocal_approx_fast(rcp[:, :, :width],
                                                 ps_r[:, :, :width])
            else:
                bden = att_sb.tile([128, 2, lc_size], F32, tag="bden", name="bden")
                nc.gpsimd.partition_broadcast(bden[:, :, :width],
                                              den[0:1, :, :width])
                nc.vector.reciprocal_approx_fast(rcp[:, :, :width],
                                                 bden[:, :, :width])
            oT = att_sb.tile([128, lc_size], BF16, tag="oT", name="oT", bufs=6)
            for h in range(HPC):
                hsl = slice(64 * h, 64 * (h + 1))
                nc.vector.tensor_mul(oT[hsl, :width], avs_h[h][:HEAD_DIM, :width],
                                     rcp[:HEAD_DIM, h, :width])
                nc.vector.tensor_scalar_add(oT[hsl, :width], oT[hsl, :width],
                                            bv_sb[hsl, :])
            return oT[:, :width]

        def oproj_lt(oT, bi, loff, lt):
            """One 128-wide l-tile of the output projection."""
            ps_o = big_ps.tile([128, 2, 512], F32, tag="big", name="ps_o")
            for dh in range(2):
                nc.tensor.matmul(ps_o[:, dh, :],
                                 oT[:, 128 * lt : 128 * (lt + 1)],
                                 wo_sb[:, 512 * dh : 512 * (dh + 1)],
                                 start=True, stop=True)
            ob = out_pool.tile([128, D_MODEL], F32, tag="ob")
            nc.vector.tensor_copy(ob[:], ps_o.rearrange("p a b -> p (a b)"))
            nc.sync.dma_start(
                out[bi * Lb + loff + 128 * lt
                    : bi * Lb + loff + 128 * (lt + 1), :], ob[:])

        # ---- static schedule ----
        chunks = [(bi, lc * lc_size) for bi in range(B) for lc in range(n_lc)]
        # scores pairs of later chunks computed during the head phase, their
        # exps parked in SBUF; ordered so each steal's qT chunk is projected
        # before it's emitted (qk_proj(c) runs at head pair t = 2c - 1).
        steals = [(1, 0), (2, 0), (1, 1), (3, 0), (2, 1), (3, 1), (1, 2), (2, 2)]
        parked = {}  # (chunk_idx, pair) -> parked ex tile
        # PE filler work inside chunk bodies: chunk_idx -> [(pair_slot,
        # kind, proj_chunk)]. v follows qk by >= one chunk so the x tiles
        # stay resident only briefly.
        fillers = {
            1: [(2, "qk", 4), (5, "qk", 5)],
            2: [(2, "v", 4), (5, "qk", 6)],
            3: [(2, "v", 5), (5, "qk", 7)],
            4: [(2, "v", 6), (4, "v", 7)],
        }
        xts_map = {}

        def run_filler(kind, c):
            if kind == "qk":
                xts_map[c] = qk_proj(c)
            else:
                v_proj(c, xts_map.pop(c))

        pending = None  # previous chunk's (avs_h, bi, loff, width) to norm
        oproj_q = []  # (oT, bi, loff, next_lt) chunks mid o-projection

        def oproj_step(all_remaining=False):
            while oproj_q:
                oT, bi, loff, lt = oproj_q[0]
                oproj_lt(oT, bi, loff, lt)
                if lt + 1 < oT.shape[-1] // 128:
                    oproj_q[0] = (oT, bi, loff, lt + 1)
                else:
                    oproj_q.pop(0)
                if not all_remaining:
                    return

        # ---- head: batch-0 projections pipelined with chunk 0 attention ----
        qk_proj(0, xts0, scalar_bias=True)
        steal_q = list(steals)
        lsl0 = slice(0, lc_size)
        ps_av = [av_ps.tile([HEAD_DIM + 1, lc_size], F32, tag=f"av{h}",
                            name=f"av{h}") for h in range(HPC)]
        exs = {}
        xts_head = {0: xts0}
        for t in range(n_pair):
            ex = exp_pool.tile([128, HPC, 2, lc_size], BF16, tag="ex", name="ex")
            sc_pair(0, lsl0, lc_size, t, ex)
            exs[t] = ex
            if t == 0:
                v_proj(0, xts_head.pop(0))
            if t in (1, 3, 5):
                c = (t + 1) // 2
                xts_head[c] = qk_proj(c)
            if t in (2, 4, 6):
                v_proj(t // 2, xts_head.pop(t // 2))
            if t >= 2 and steal_q:
                cj, p = steal_q.pop(0)
                pex = pre_pool.tile([128, HPC, 2, lc_size], BF16,
                                    tag=f"pre{cj}_{p}", name=f"pre{cj}_{p}")
                bj, loffj = chunks[cj]
                sc_pair(bj, slice(bj * Lb + loffj, bj * Lb + loffj + lc_size),
                        lc_size, p, pex)
                parked[(cj, p)] = pex
            if t >= 1:
                av_pair(0, ps_av, t - 1, exs.pop(t - 1), lc_size,
                        start=(t - 1 == 0), stop=False)
        while steal_q:
            cj, p = steal_q.pop(0)
            pex = pre_pool.tile([128, HPC, 2, lc_size], BF16,
                                tag=f"pre{cj}_{p}", name=f"pre{cj}_{p}")
            bj, loffj = chunks[cj]
            sc_pair(bj, slice(bj * Lb + loffj, bj * Lb + loffj + lc_size),
                    lc_size, p, pex)
            parked[(cj, p)] = pex
        av_pair(0, ps_av, n_pair - 1, exs.pop(n_pair - 1), lc_size,
                start=False, stop=True)
        avs = att_sb.tile([HEAD_DIM + 1, 2, lc_size], F32, tag="avs", name="avs")
        for h in range(HPC):
            nc.vector.tensor_copy(avs[:, h, :], ps_av[h][:])
        pending = ([avs[:, h, :] for h in range(HPC)], 0, 0, lc_size)

        # ---- remaining chunks ----
        for ci in range(1, len(chunks)):
            bi, loff = chunks[ci]
            width = lc_size
            lsl = slice(bi * Lb + loff, bi * Lb + loff + width)
            ps_av = [av_ps.tile([HEAD_DIM + 1, lc_size], F32, tag=f"av{h}",
                                name=f"av{h}") for h in range(HPC)]
            exs = {}
            body_fill = dict((slot, (kind, c))
                             for slot, kind, c in fillers.get(ci, []))
            last = ci == len(chunks) - 1
            for t in range(n_pair):
                if (ci, t) in parked:
                    exs[t] = parked.pop((ci, t))
                else:
                    ex = exp_pool.tile([128, HPC, 2, lc_size], BF16,
                                       tag="ex", name="ex")
                    sc_pair(bi, lsl, width, t, ex)
                    exs[t] = ex
                if t == 1 and pending is not None:
                    oproj_q.append((norm_part(pending[0], pending[3]),)
                                   + pending[1:3] + (0,))
                    pending = None
                if t in body_fill:
                    run_filler(*body_fill[t])
                elif t >= 3:
                    # one o-proj l-tile between score pairs; a single block
                    # would starve ScalarE via the PSUM-pool rotation
                    oproj_step()
                if t >= 1:
                    av_pair(bi, ps_av, t - 1, exs.pop(t - 1), width,
                            start=(t - 1 == 0), stop=False)
            av_pair(bi, ps_av, n_pair - 1, exs.pop(n_pair - 1), width,
                    start=False, stop=True)
            if last:
                # final chunk: normalize straight from PSUM with the PE idle
                pending = ([ps_av[h][:, :width] for h in range(HPC)],
                           bi, loff, width)
            else:
                avs = att_sb.tile([HEAD_DIM + 1, 2, lc_size], F32, tag="avs",
                                  name="avs")
                for h in range(HPC):
                    nc.vector.tensor_copy(avs[:, h, :width], ps_av[h][:, :width])
                pending = ([avs[:, h, :] for h in range(HPC)], bi, loff, width)

        oproj_step(all_remaining=True)
        oproj_q.append((norm_part(pending[0], pending[3], pe_bcast=True),)
                       + pending[1:3] + (0,))
        oproj_step(all_remaining=True)

    nc.compile()
    return nc


def make_in_maps(x, Wq, bq, Wk, bk, Wv, bv, Wo, Lb=L):
    """Per-core input dicts from full inputs."""
    BLb = B * Lb
    xT = np.ascontiguousarray(
        np.asarray(x, np.float32).reshape(BLb, D_MODEL).T).astype(NPBF16)
    Wq = np.asarray(Wq, np.float32).astype(NPBF16)
    Wk = np.asarray(Wk, np.float32).astype(NPBF16)
    Wv = np.asarray(Wv, np.float32).astype(NPBF16)
    Wo = np.asarray(Wo, np.float32).astype(NPBF16)
    in_maps = []
    for c in range(N_CORES):
        dsl = slice(MLOC * c, MLOC * (c + 1))
        in_maps.append({
            "xT": xT,
            "wq": np.ascontiguousarray(Wq[:, dsl]),
            "wk": np.ascontiguousarray(Wk[:, dsl]),
            "wv": np.ascontiguousarray(Wv[:, dsl]),
            "wo": np.ascontiguousarray(Wo[dsl, :]),
            "bq": np.ascontiguousarray(np.asarray(bq, np.float32)[dsl].reshape(MLOC, 1)),
            "bk": np.ascontiguousarray(np.asarray(bk, np.float32)[dsl].reshape(MLOC, 1)),
            "bv": np.ascontiguousarray(np.asarray(bv, np.float32)[dsl].reshape(MLOC, 1)),
        })
    return in_maps


_NC_CACHE = {}


def _get_nc():
    if "nc" not in _NC_CACHE:
        _NC_CACHE["nc"] = build_nc()
    return _NC_CACHE["nc"]


def kernel(x, Wq, bq, Wk, bk, Wv, bv, Wo, bo):
    nc = _get_nc()
    in_maps = make_in_maps(x, Wq, bq, Wk, bk, Wv, bv, Wo)
    res = run_bass_kernel_spmd(nc, in_maps, list(range(N_CORES)))
    acc = np.zeros((B * L, D_MODEL), dtype=np.float32)
    for c in range(N_CORES):
        acc += res.results[c]["out"]
    acc += np.asarray(bo, dtype=np.float32)
    return acc.reshape(B, L, D_MODEL)


# revision 26
# speedup vs baseline: 1.1660x; 1.1660x over previous
"""Multihead attention (B=2, L=2048, D=1024, 16 heads) on 8 trn2 cores.

Sharding: tensor-parallel over heads — 2 heads per core. Each core computes
q/k/v projections for its 128 columns of Wq/Wk/Wv, full attention for its two
heads, and a partial output projection against its 128 rows of Wo. The host
sums the 8 partials and adds bo.

Engine budget per core: ScalarE exp over B*HPC*L^2 = 16.8M scores is ~147us
of ACTIVATE; PE is ~170us of bf16 matmul. The schedule keeps both dense:

  - Warmup matmuls at kernel start trip the PE HAM clock-gate out of its
    cold 1.2GHz state while the first x tiles stream in (DMA spread over
    sync/scalar/vector queues; weight tensors on gpsimd/vector/scalar so
    wv is not serialized behind wq/wk).
  - Head phase: batch-0 projections are software-pipelined with the first
    l-chunk's attention, and score pairs of LATER l-chunks are "stolen" into
    the head (exps parked in SBUF) so ScalarE never starves while the PE
    grinds projections. Chunk-0 q/k bias-adds run on ScalarE (idle then).
  - scoresT packs both heads into one PE pass via 64-row tile_position;
    attn@v accumulates both s-tiles of a pair in PSUM with the softmax
    denominator as a 65th ones-column of v.
  - o-projection of the previous chunk is spread one l-tile at a time
    between score pairs (a single block starves ScalarE via PSUM-pool
    rotation); batch-1 projections fill batch-0's chunks the same way.
  - The tail normalize uses a PE broadcast (PE idle there) straight from
    attn@v PSUM.
"""

from contextlib import ExitStack

import ml_dtypes
import numpy as np

import concourse.bacc as bacc
import concourse.mybir as mybir
import concourse.tile as tile
from concourse.bass_utils import run_bass_kernel_spmd

D_MODEL = 1024
N_HEAD = 16
HEAD_DIM = 64
B = 2
L = 2048
N_CORES = 8
HPC = N_HEAD // N_CORES  # heads per core
MLOC = HPC * HEAD_DIM  # 128: local d width per core

F32 = mybir.dt.float32
BF16 = mybir.dt.bfloat16
NPBF16 = ml_dtypes.bfloat16


def build_nc(Lb=L, lc_size=512, nch=512):
    """Build the per-core Bass program. Lb = sequence length per batch."""
    BLb = B * Lb
    KT = D_MODEL // 128  # 8 contraction tiles for the projections
    n_nch = BLb // nch  # projection column chunks (8; 0-3 = b0, 4-7 = b1)
    n_lc = Lb // lc_size  # attention l-chunks per batch
    n_st = Lb // 128  # s-tiles per batch
    n_pair = n_st // 2  # s-tile pairs per batch

    nc = bacc.Bacc("TRN2", target_bir_lowering=False, debug=False)

    xT = nc.dram_tensor("xT", [D_MODEL, BLb], BF16, kind="ExternalInput").ap()
    wq = nc.dram_tensor("wq", [D_MODEL, MLOC], BF16, kind="ExternalInput").ap()
    wk = nc.dram_tensor("wk", [D_MODEL, MLOC], BF16, kind="ExternalInput").ap()
    wv = nc.dram_tensor("wv", [D_MODEL, MLOC], BF16, kind="ExternalInput").ap()
    wo = nc.dram_tensor("wo", [MLOC, D_MODEL], BF16, kind="ExternalInput").ap()
    bq = nc.dram_tensor("bq", [MLOC, 1], F32, kind="ExternalInput").ap()
    bk = nc.dram_tensor("bk", [MLOC, 1], F32, kind="ExternalInput").ap()
    bv = nc.dram_tensor("bv", [MLOC, 1], F32, kind="ExternalInput").ap()
    out = nc.dram_tensor("out", [BLb, D_MODEL], F32, kind="ExternalOutput").ap()

    with tile.TileContext(nc) as tc, ExitStack() as ctx:
        consts = ctx.enter_context(tc.tile_pool(name="consts", bufs=1))
        qk_sb = ctx.enter_context(tc.tile_pool(name="qk_sb", bufs=1))
        xt_pool = ctx.enter_context(tc.tile_pool(name="xt", bufs=32))
        # PSUM: big pool (2-bank slots x3) shared by scoresT / projections /
        # o-proj; av pool: one 1-bank tile per head. Total 8 banks.
        big_ps = ctx.enter_context(tc.tile_pool(name="big_ps", bufs=3, space="PSUM"))
        av_ps = ctx.enter_context(tc.tile_pool(name="av_ps", bufs=1, space="PSUM"))
        exp_pool = ctx.enter_context(tc.tile_pool(name="expT", bufs=4))
        pre_pool = ctx.enter_context(tc.tile_pool(name="preT", bufs=1))
        att_sb = ctx.enter_context(tc.tile_pool(name="att_sb", bufs=3))
        out_pool = ctx.enter_context(tc.tile_pool(name="out_sb", bufs=6))

        # PE warmup: ~4us of junk matmuls (no input deps) so the HAM clock
        # gate reaches 2.4GHz before the first real projection matmuls.
        warm = consts.tile([1, 128], F32, tag="warm")
        nc.vector.memset(warm[:], 0.0)
        ps_warm = big_ps.tile([128, 2, nch], F32, tag="big", name="ps_warm")
        for _ in range(10):
            nc.tensor.matmul(ps_warm[:, 0, :128], warm[:], warm[:],
                             start=True, stop=True)

        def load_xts(nc_i, first=False):
            csl = slice(nc_i * nch, (nc_i + 1) * nch)
            xts = []
            for k in range(KT):
                xt = xt_pool.tile([128, nch], BF16, tag="xt", name="xt")
                # scalar-queue DMAs steal ScalarE issue slots from ACTIVATE;
                # only the first chunk (pre-exp) may use the scalar queue
                eng = (nc.sync if k % 2 == 0 else nc.scalar) if first else nc.sync
                eng.dma_start(xt[:], xT[128 * k : 128 * (k + 1), csl])
                xts.append(xt)
            return xts

        # First projection chunk's x tiles before anything else so the PE
        # starts as early as possible.
        xts0 = load_xts(0, first=True)

        # Weights + biases spread over the three DMA queues so nothing that
        # gates the first scores (bq/bk!) or first attn@v (wv) queues behind
        # the bulk weight transfers: gpsimd = biases + wq + wo, sync = x
        # evens + wk, scalar = x odds + wv.
        bq_sb = consts.tile([MLOC, 1], F32, tag="bq")
        bk_sb = consts.tile([MLOC, 1], F32, tag="bk")
        bv_sb = consts.tile([MLOC, 1], F32, tag="bv")
        for b_sb, b_dram in ((bq_sb, bq), (bk_sb, bk), (bv_sb, bv)):
            nc.gpsimd.dma_start(b_sb[:], b_dram)
        wq_sb = consts.tile([128, KT, MLOC], BF16, tag="wq")
        wk_sb = consts.tile([128, KT, MLOC], BF16, tag="wk")
        wv_sb = consts.tile([128, KT, MLOC], BF16, tag="wv")
        wo_sb = consts.tile([128, D_MODEL], BF16, tag="wo")
        for eng, w_sb, w_dram in ((nc.gpsimd, wq_sb, wq), (nc.sync, wk_sb, wk),
                                  (nc.scalar, wv_sb, wv)):
            wr = w_dram.rearrange("(k p) m -> p k m", p=128)
            for k in range(KT):
                eng.dma_start(w_sb[:, k, :], wr[:, k, :])
        nc.gpsimd.dma_start(wo_sb[:], wo)
        ones_f32 = consts.tile([1, 128], F32, tag="ones_f32")
        nc.vector.memset(ones_f32[:], 1.0)

        # Persistent activations.
        qT_sb = qk_sb.tile([128, BLb], BF16, tag="qT")  # [d_local, b*Lb+l]
        kT_sb = qk_sb.tile([128, BLb], BF16, tag="kT")
        # v (natural layout) + ones column: per (b, head): [128, n_st, 65]
        vaug = [
            [qk_sb.tile([128, n_st, HEAD_DIM + 1], BF16, tag=f"vaug{bi}{h}",
                        name=f"vaug{bi}{h}")
             for h in range(HPC)]
            for bi in range(B)
        ]
        for bi in range(B):
            for h in range(HPC):
                nc.vector.memset(vaug[bi][h][:, :, HEAD_DIM:], 1.0)

        def qk_proj(nc_i, xts=None, scalar_bias=False):
            """q/k projections for one column chunk of x. Returns xts for the
            later v_proj of the same chunk."""
            csl = slice(nc_i * nch, (nc_i + 1) * nch)
            if xts is None:
                xts = load_xts(nc_i)
            ps_qk = big_ps.tile([128, 2, nch], F32, tag="big", name="ps_qk")
            for k in range(KT):
                nc.tensor.matmul(ps_qk[:, 0, :], wq_sb[:, k, :], xts[k][:],
                                 start=(k == 0), stop=(k == KT - 1))
                nc.tensor.matmul(ps_qk[:, 1, :], wk_sb[:, k, :], xts[k][:],
                                 start=(k == 0), stop=(k == KT - 1))
            if scalar_bias:
                # ScalarE is idle during the head; keep DVE off the critical
                # path to the first scores.
                nc.scalar.activation(qT_sb[:, csl], ps_qk[:, 0, :],
                                     mybir.ActivationFunctionType.Identity,
                                     bias=bq_sb[:])
                nc.scalar.activation(kT_sb[:, csl], ps_qk[:, 1, :],
                                     mybir.ActivationFunctionType.Identity,
                                     bias=bk_sb[:])
            else:
                nc.vector.tensor_scalar_add(qT_sb[:, csl], ps_qk[:, 0, :],
                                            bq_sb[:])
                nc.vector.tensor_scalar_add(kT_sb[:, csl], ps_qk[:, 1, :],
                                            bk_sb[:])
            return xts

        def v_proj(nc_i, xts):
            """v projection (natural [s, d] layout) for one chunk."""
            st_per_nch = nch // 128
            ps_v = big_ps.tile([128, nch], F32, tag="big", name="ps_v")
            for st in range(st_per_nch):
                ssl = slice(128 * st, 128 * (st + 1))
                for k in range(KT):
                    nc.tensor.matmul(ps_v[:, ssl], xts[k][:, ssl],
                                     wv_sb[:, k, :],
                                     start=(k == 0), stop=(k == KT - 1))
            for st in range(st_per_nch):
                st_g = nc_i * st_per_nch + st
                bi, st_b = divmod(st_g, n_st)
                for h in range(HPC):
                    nc.vector.tensor_copy(
                        vaug[bi][h][:, st_b, :HEAD_DIM],
                        ps_v[:, 128 * st + HEAD_DIM * h
                             : 128 * st + HEAD_DIM * (h + 1)])

        def sc_pair(bi, lsl, width, t, ex):
            """scoresT + exp for s-tile pair t of one l-chunk, into bf16 ex
            tile [128, HPC, 2, lc_size]."""
            for i in range(2):
                st = 2 * t + i
                ssl = slice(bi * Lb + st * 128, bi * Lb + (st + 1) * 128)
                ps_sc = big_ps.tile([128, HPC, lc_size], F32, tag="big",
                                    name="ps_sc")
                for h in range(HPC):
                    hsl = slice(64 * h, 64 * (h + 1))
                    nc.tensor.matmul(ps_sc[:, h, :width], kT_sb[hsl, ssl],
                                     qT_sb[hsl, lsl],
                                     start=True, stop=True,
                                     tile_position=(64 * h, 0))
                nc.scalar.activation(ex[:, :, i, :width], ps_sc[:, :, :width],
                                     mybir.ActivationFunctionType.Exp,
                                     scale=1.0 / np.sqrt(HEAD_DIM))

        def av_pair(bi, ps_av, t, ex, width, start, stop):
            """attn@v for s-tile pair t (both s-tiles, both heads)."""
            for i in range(2):
                st = 2 * t + i
                for h in range(HPC):
                    nc.tensor.matmul(ps_av[h][:, :width],
                                     vaug[bi][h][:, st, :],
                                     ex[:, h, i, :width],
                                     start=start and i == 0,
                                     stop=stop and i == 1)

        def norm_part(avs_h, width, pe_bcast=False):
            """Normalization chain -> oT (bf16 lhsT for o-proj). avs_h is a
            per-head list of [65, width] APs (SBUF copies, or PSUM directly
            for the final chunk). pe_bcast uses a K=1 PE matmul for the
            denominator broadcast (for the tail, where the PE is idle)."""
            den = att_sb.tile([1, 2, lc_size], F32, tag="den", name="den")
            rcp = att_sb.tile([128, 2, lc_size], F32, tag="rcp", name="rcp")
            for h in range(HPC):
                nc.vector.tensor_copy(den[0:1, h, :width], avs_h[h][64:65, :width])
            if pe_bcast:
                ps_r = big_ps.tile([128, 2, lc_size], F32, tag="big", name="ps_r")
                for h in range(HPC):
                    nc.tensor.matmul(ps_r[:, h, :width], ones_f32[:],
                                     den[0:1, h, :width], start=True, stop=True)
                nc.vector.reciprocal_approx_fast(rcp[:, :, :width],
                                                 ps_r[:, :, :width])
            else:
                bden = att_sb.tile([128, 2, lc_size], F32, tag="bden", name="bden")
                nc.gpsimd.partition_broadcast(bden[:, :, :width],
                                              den[0:1, :, :width])
                nc.vector.reciprocal_approx_fast(rcp[:, :, :width],
                                                 bden[:, :, :width])
            oT = att_sb.tile([128, lc_size], BF16, tag="oT", name="oT", bufs=6)
            for h in range(HPC):
                hsl = slice(64 * h, 64 * (h + 1))
                nc.vector.tensor_mul(oT[hsl, :width], avs_h[h][:HEAD_DIM, :width],
                                     rcp[:HEAD_DIM, h, :width])
                nc.vector.tensor_scalar_add(oT[hsl, :width], oT[hsl, :width],
                                            bv_sb[hsl, :])
            return oT[:, :width]

        def oproj_lt(oT, bi, loff, lt):
            """One 128-wide l-tile of the output projection."""
            ps_o = big_ps.tile([128, 2, 512], F32, tag="big", name="ps_o")
            for dh in range(2):
                nc.tensor.matmul(ps_o[:, dh, :],
                                 oT[:, 128 * lt : 128 * (lt + 1)],
                                 wo_sb[:, 512 * dh : 512 * (dh + 1)],
                                 start=True, stop=True)
            ob = out_pool.tile([128, D_MODEL], F32, tag="ob")
            nc.vector.tensor_copy(ob[:], ps_o.rearrange("p a b -> p (a b)"))
            # out stores ride gpsimd: on sync they head-of-line block the
            # filler x tiles behind their (DVE-dependent) evacuation copies
            nc.gpsimd.dma_start(
                out[bi * Lb + loff + 128 * lt
                    : bi * Lb + loff + 128 * (lt + 1), :], ob[:])

        # ---- static schedule ----
        chunks = [(bi, lc * lc_size) for bi in range(B) for lc in range(n_lc)]
        # scores pairs of later chunks computed during the head phase, their
        # exps parked in SBUF; ordered so each steal's qT chunk is projected
        # before it's emitted (qk_proj(c) runs at head pair t = 2c - 1).
        steals = [(1, 0), (2, 0), (1, 1), (3, 0), (2, 1), (3, 1), (1, 2), (2, 2)]
        parked = {}  # (chunk_idx, pair) -> parked ex tile
        # PE filler work inside chunk bodies: chunk_idx -> [(pair_slot,
        # kind, proj_chunk)]. v follows qk by >= one chunk so the x tiles
        # stay resident only briefly.
        fillers = {
            1: [(2, "qk", 4), (5, "qk", 5)],
            2: [(2, "v", 4), (5, "qk", 6)],
            3: [(2, "v", 5), (5, "qk", 7)],
            4: [(2, "v", 6), (4, "v", 7)],
        }
        xts_map = {}

        def run_filler(kind, c):
            if kind == "qk":
                xts_map[c] = qk_proj(c)
            else:
                v_proj(c, xts_map.pop(c))

        pending = None  # previous chunk's (avs_h, bi, loff, width) to norm
        oproj_q = []  # (oT, bi, loff, next_lt) chunks mid o-projection

        def oproj_step(all_remaining=False):
            while oproj_q:
                oT, bi, loff, lt = oproj_q[0]
                oproj_lt(oT, bi, loff, lt)
                if lt + 1 < oT.shape[-1] // 128:
                    oproj_q[0] = (oT, bi, loff, lt + 1)
                else:
                    oproj_q.pop(0)
                if not all_remaining:
                    return

        # ---- head: batch-0 projections pipelined with chunk 0 attention ----
        qk_proj(0, xts0, scalar_bias=True)
        steal_q = list(steals)
        lsl0 = slice(0, lc_size)
        ps_av = [av_ps.tile([HEAD_DIM + 1, lc_size], F32, tag=f"av{h}",
                            name=f"av{h}") for h in range(HPC)]
        exs = {}
        xts_head = {0: xts0}
        for t in range(n_pair):
            ex = exp_pool.tile([128, HPC, 2, lc_size], BF16, tag="ex", name="ex")
            sc_pair(0, lsl0, lc_size, t, ex)
            exs[t] = ex
            if t == 0:
                v_proj(0, xts_head.pop(0))
            if t in (1, 3, 5):
                c = (t + 1) // 2
                xts_head[c] = qk_proj(c)
            if t in (2, 4, 6):
                v_proj(t // 2, xts_head.pop(t // 2))
            if t >= 2 and steal_q:
                cj, p = steal_q.pop(0)
                pex = pre_pool.tile([128, HPC, 2, lc_size], BF16,
                                    tag=f"pre{cj}_{p}", name=f"pre{cj}_{p}")
                bj, loffj = chunks[cj]
                sc_pair(bj, slice(bj * Lb + loffj, bj * Lb + loffj + lc_size),
                        lc_size, p, pex)
                parked[(cj, p)] = pex
            if t >= 1:
                av_pair(0, ps_av, t - 1, exs.pop(t - 1), lc_size,
                        start=(t - 1 == 0), stop=False)
        while steal_q:
            cj, p = steal_q.pop(0)
            pex = pre_pool.tile([128, HPC, 2, lc_size], BF16,
                                tag=f"pre{cj}_{p}", name=f"pre{cj}_{p}")
            bj, loffj = chunks[cj]
            sc_pair(bj, slice(bj * Lb + loffj, bj * Lb + loffj + lc_size),
                    lc_size, p, pex)
            parked[(cj, p)] = pex
        av_pair(0, ps_av, n_pair - 1, exs.pop(n_pair - 1), lc_size,
                start=False, stop=True)
        avs = att_sb.tile([HEAD_DIM + 1, 2, lc_size], F32, tag="avs", name="avs")
        for h in range(HPC):
            nc.vector.tensor_copy(avs[:, h, :], ps_av[h][:])
        pending = ([avs[:, h, :] for h in range(HPC)], 0, 0, lc_size)

        # ---- remaining chunks ----
        for ci in range(1, len(chunks)):
            bi, loff = chunks[ci]
            width = lc_size
            lsl = slice(bi * Lb + loff, bi * Lb + loff + width)
            ps_av = [av_ps.tile([HEAD_DIM + 1, lc_size], F32, tag=f"av{h}",
                                name=f"av{h}") for h in range(HPC)]
            exs = {}
            body_fill = dict((slot, (kind, c))
                             for slot, kind, c in fillers.get(ci, []))
            last = ci == len(chunks) - 1
            for t in range(n_pair):
                if (ci, t) in parked:
                    exs[t] = parked.pop((ci, t))
                else:
                    ex = exp_pool.tile([128, HPC, 2, lc_size], BF16,
                                       tag="ex", name="ex")
                    sc_pair(bi, lsl, width, t, ex)
                    exs[t] = ex
                if t == 1 and pending is not None:
                    oproj_q.append((norm_part(pending[0], pending[3]),)
                                   + pending[1:3] + (0,))
                    pending = None
                if t in body_fill:
                    run_filler(*body_fill[t])
                elif t >= 3:
                    # one o-proj l-tile between score pairs; a single block
                    # would starve ScalarE via the PSUM-pool rotation
                    oproj_step()
                if t >= 1:
                    av_pair(bi, ps_av, t - 1, exs.pop(t - 1), width,
                            start=(t - 1 == 0), stop=False)
            av_pair(bi, ps_av, n_pair - 1, exs.pop(n_pair - 1), width,
                    start=False, stop=True)
            if last:
                # final chunk: normalize straight from PSUM with the PE idle
                pending = ([ps_av[h][:, :width] for h in range(HPC)],
                           bi, loff, width)
            else:
                avs = att_sb.tile([HEAD_DIM + 1, 2, lc_size], F32, tag="avs",
                                  name="avs")
                for h in range(HPC):
                    nc.vector.tensor_copy(avs[:, h, :width], ps_av[h][:, :width])
                pending = ([avs[:, h, :] for h in range(HPC)], bi, loff, width)

        oproj_step(all_remaining=True)
        oproj_q.append((norm_part(pending[0], pending[3], pe_bcast=True),)
                       + pending[1:3] + (0,))
        oproj_step(all_remaining=True)

    nc.compile()
    return nc


def make_in_maps(x, Wq, bq, Wk, bk, Wv, bv, Wo, Lb=L):
    """Per-core input dicts from full inputs."""
    BLb = B * Lb
    xT = np.ascontiguousarray(
        np.asarray(x, np.float32).reshape(BLb, D_MODEL).T).astype(NPBF16)
    Wq = np.asarray(Wq, np.float32).astype(NPBF16)
    Wk = np.asarray(Wk, np.float32).astype(NPBF16)
    Wv = np.asarray(Wv, np.float32).astype(NPBF16)
    Wo = np.asarray(Wo, np.float32).astype(NPBF16)
    in_maps = []
    for c in range(N_CORES):
        dsl = slice(MLOC * c, MLOC * (c + 1))
        in_maps.append({
            "xT": xT,
            "wq": np.ascontiguousarray(Wq[:, dsl]),
            "wk": np.ascontiguousarray(Wk[:, dsl]),
            "wv": np.ascontiguousarray(Wv[:, dsl]),
            "wo": np.ascontiguousarray(Wo[dsl, :]),
            "bq": np.ascontiguousarray(np.asarray(bq, np.float32)[dsl].reshape(MLOC, 1)),
            "bk": np.ascontiguousarray(np.asarray(bk, np.float32)[dsl].reshape(MLOC, 1)),
            "bv": np.ascontiguousarray(np.asarray(bv, np.float32)[dsl].reshape(MLOC, 1)),
        })
    return in_maps


_NC_CACHE = {}


def _get_nc():
    if "nc" not in _NC_CACHE:
        _NC_CACHE["nc"] = build_nc()
    return _NC_CACHE["nc"]


def kernel(x, Wq, bq, Wk, bk, Wv, bv, Wo, bo):
    nc = _get_nc()
    in_maps = make_in_maps(x, Wq, bq, Wk, bk, Wv, bv, Wo)
    res = run_bass_kernel_spmd(nc, in_maps, list(range(N_CORES)))
    acc = np.zeros((B * L, D_MODEL), dtype=np.float32)
    for c in range(N_CORES):
        acc += res.results[c]["out"]
    acc += np.asarray(bo, dtype=np.float32)
    return acc.reshape(B, L, D_MODEL)
